# revision 23
# baseline (speedup 1.0000x reference)
"""Trainium2 Bass kernel for nn_BioMoR_RNN (GRU recurrence + small output heads).

Strategy (pure data parallel, per the spec sharding hint):
  - batch 128 is split 16-per-core across 8 NeuronCores; all weights replicated.
  - The GRU recurrence runs gate-major: per step, 18 bf16 matmuls accumulate
    W_hh @ h plus the x/bias contributions into PSUM tiles [gate, batch] for
    r, z, n.  The x / bias matmuls have no h dependency and OPEN each PSUM
    accumulation group (start=True), so the scheduler executes them during the
    previous step's nonlinear chain; only the 12 W_hh matmuls sit on the
    critical path, issuing back-to-back (~27ns) with preloaded weights.
  - Chain per step: sigmoid(r) overlaps the n/z matmuls; n-path is
    mul(psum_n, r) -> +gi_n -> tanh; update is h' = (1-z)*n + z*h with
    z*h and (1-z) computed during the tanh window, leaving only two
    dependent vector ops after tanh.  All elementwise ops are bf16 (DVE 2x).
  - The n-gate x contribution (gi_n) is precomputed per 64-step chunk as
    batched K=7 matmuls; b_hh_n enters psum_n via a K=1 ones-row matmul.
  - Output heads are computed as batched matmuls over 128-sample slices,
    interleaved between every 8 steps so they fill engine idle time without
    head-of-line blocking the recurrence.  The h@w_bias head accumulates into
    the same PSUM tile as the policy/motor matmuls (at partition bases 0/32),
    so probs/dirs each collapse to one activation op with constant bias;
    b_reflex enters the feat psum via K=1 ones-matmuls (single wide relu).
  - h_t streams to HBM as [h, s, b] (gate-major, bf16); the host transposes
    and casts.  Dynamic loop over chunks (8 per body) keeps code size small.

Self-contained: hardcodes shapes from the problem spec; no file reads.
"""

import contextlib
import numpy as np
import ml_dtypes

B, S_FULL, H, DX = 128, 2048, 256, 6
NCORES = 8
BL = B // NCORES  # 16 batch rows per core
CHUNK = 64        # steps per chunk

_CACHE = {}


def _build(S=S_FULL, state_fp32=True, debug=False):
    """Build + schedule the per-core kernel (same NEFF for all 8 cores)."""
    import concourse.tile as tile
    from concourse import bacc, mybir
    from concourse.bass import ds, ts

    f32 = mybir.dt.float32
    bf16 = mybir.dt.bfloat16
    Alu = mybir.AluOpType
    Act = mybir.ActivationFunctionType
    wdt = bf16

    NCH = S // CHUNK
    assert S % CHUNK == 0 and NCH % 2 == 0
    hdt = f32 if state_fp32 else bf16

    nc = bacc.Bacc("TRN2", target_bir_lowering=False, debug=debug)

    x_d = nc.dram_tensor("x", [7, (NCH + 1) * CHUNK * BL], wdt, kind="ExternalInput")
    whh_d = nc.dram_tensor("whh", [128, 2 * 768], wdt, kind="ExternalInput")
    wx_d = nc.dram_tensor("wx", [7, 768], wdt, kind="ExternalInput")
    bhn_d = nc.dram_tensor("bhn", [1, 256], wdt, kind="ExternalInput")
    wgb_d = nc.dram_tensor("wgb", [128, 4], wdt, kind="ExternalInput")
    wbm_d = nc.dram_tensor("wbm", [128, 68], wdt, kind="ExternalInput")
    wrx_d = nc.dram_tensor("wrx", [2, 256], wdt, kind="ExternalInput")
    wpm_d = nc.dram_tensor("wpm", [128, 68], wdt, kind="ExternalInput")
    bgain_d = nc.dram_tensor("bgain", [2, 1], f32, kind="ExternalInput")
    bgb_d = nc.dram_tensor("bgb", [2, 2], f32, kind="ExternalInput")
    brx_d = nc.dram_tensor("brx", [1, 256], wdt, kind="ExternalInput")
    h_d = nc.dram_tensor("h_out", [128, 2, S, BL], hdt, kind="ExternalOutput")
    y_d = nc.dram_tensor("y_out", [4, S, BL], f32, kind="ExternalOutput")

    with tile.TileContext(nc) as tc, contextlib.ExitStack() as ctx:
        constp = ctx.enter_context(tc.tile_pool(name="const", bufs=1))
        przp = ctx.enter_context(tc.tile_pool(name="prz", bufs=2, space="PSUM"))
        pzp = ctx.enter_context(tc.tile_pool(name="pz", bufs=1, space="PSUM"))
        pnp = ctx.enter_context(tc.tile_pool(name="pn", bufs=2, space="PSUM"))
        phd = ctx.enter_context(tc.tile_pool(name="phd", bufs=2, space="PSUM"))
        pgi = ctx.enter_context(tc.tile_pool(name="pgi", bufs=1, space="PSUM"))
        rzp = ctx.enter_context(tc.tile_pool(name="rzp", bufs=4))
        tmpp = ctx.enter_context(tc.tile_pool(name="tmp", bufs=4))

        whh = constp.tile([128, 2 * 768], wdt, tag="whh")
        wx = constp.tile([7, 768], wdt, tag="wx")
        bhn = constp.tile([1, 256], wdt, tag="bhn")
        ones16 = constp.tile([1, 128], wdt, tag="ones16")
        wgb = constp.tile([128, 4], wdt, tag="wgb")
        wbm = constp.tile([128, 68], wdt, tag="wbm")
        wrx = constp.tile([2, 256], wdt, tag="wrx")
        wpm = constp.tile([128, 68], wdt, tag="wpm")
        bgain = constp.tile([2, 1], f32, tag="bgain")
        bgb = constp.tile([2, 2], f32, tag="bgb")
        brx = constp.tile([1, 256], wdt, tag="brx")
        hbf = [constp.tile([128, 2, CHUNK, BL], wdt, tag=f"hbf{i}", name=f"hbf{i}") for i in (0, 1)]
        if state_fp32:
            hstg = [constp.tile([128, 2, CHUNK, BL], f32, tag=f"hstg{i}", name=f"hstg{i}") for i in (0, 1)]
        else:
            hstg = hbf
        xbuf = [constp.tile([7, CHUNK * BL], wdt, tag=f"xb{i}", name=f"xb{i}") for i in (0, 1)]
        gin = [constp.tile([128, CHUNK, 2, BL], wdt, tag=f"gin{i}", name=f"gin{i}") for i in (0, 1)]
        ysp = [constp.tile([2, CHUNK, BL], f32, tag=f"ysp{i}", name=f"ysp{i}") for i in (0, 1)]
        ysd = [constp.tile([2, CHUNK, BL], f32, tag=f"ysd{i}", name=f"ysd{i}") for i in (0, 1)]

        for tdst, tsrc in [(whh, whh_d), (wx, wx_d), (bhn, bhn_d), (wgb, wgb_d),
                           (wrx, wrx_d), (wpm, wpm_d), (wbm, wbm_d), (bgain, bgain_d),
                           (bgb, bgb_d), (brx, brx_d)]:
            nc.sync.dma_start(tdst[:], tsrc[:])
        nc.sync.dma_start(xbuf[0][:], x_d[:, 0:CHUNK * BL])
        nc.vector.memset(ones16[:], 1.0)
        nc.vector.memset(hbf[1][:, :, CHUNK - 1, :], 0.0)
        if state_fp32:
            nc.vector.memset(hstg[1][:, :, CHUNK - 1, :], 0.0)

        def emit_gin(dst, xsrc, pieces=(0, 1, 2, 3)):
            # n-gate x contribution (incl. b_ih_n via the ones row)
            for q in pieces:
                for mt in (0, 1):
                    pg = pgi.tile([128, 256], f32, tag="pg")
                    nc.tensor.matmul(pg[:], wx[:, (4 + mt) * 128:(5 + mt) * 128],
                                     xsrc[:, q * 256:(q + 1) * 256],
                                     start=True, stop=True)
                    nc.vector.tensor_copy(dst[:, q * 16:(q + 1) * 16, mt, :], pg[:])

        def emit_steps(p, s0, s1):
            curh, prevh = hbf[p], hbf[1 - p]
            curf, prevf = hstg[p], hstg[1 - p]
            xb, gi = xbuf[p], gin[p]
            for s in range(s0, s1):
                hsrc = curh if s else prevh
                fsrc = curf if s else prevf
                sp = s - 1 if s else CHUNK - 1
                pr = przp.tile([128, 32], f32, tag="pr", name="pr")
                pz = pzp.tile([128, 32], f32, tag="pz", name="pz")
                pn = pnp.tile([128, 32], f32, tag="pn", name="pn")
                # One accumulation group per PSUM tile: the x/bias matmul opens
                # the group (start=True clears the bank) and has no h dependency,
                # so the scheduler runs it during the previous step's chain.
                grp = {"r": [], "n": [], "z": []}
                for j, gt in enumerate((0, 1)):
                    grp["r"].append(nc.tensor.matmul(
                        pr[:, j * BL:(j + 1) * BL], wx[:, gt * 128:(gt + 1) * 128],
                        xb[:, s * BL:(s + 1) * BL], start=(j == 0), stop=False))
                for j in (0, 1):
                    grp["n"].append(nc.tensor.matmul(
                        pn[:, j * BL:(j + 1) * BL], bhn[0:1, j * 128:(j + 1) * 128],
                        ones16[0:1, 0:BL], start=(j == 0), stop=False))
                for j, gt in enumerate((2, 3)):
                    grp["z"].append(nc.tensor.matmul(
                        pz[:, j * BL:(j + 1) * BL], wx[:, gt * 128:(gt + 1) * 128],
                        xb[:, s * BL:(s + 1) * BL], start=(j == 0), stop=False))
                # h matmuls: r first (sigmoid(r) overlaps), then n, then z
                for g, base, pt in (("r", 0, pr), ("n", 4, pn), ("z", 2, pz)):
                    for j in (0, 1):
                        gt = base + j
                        o = pt[:, j * BL:(j + 1) * BL]
                        grp[g].append(nc.tensor.matmul(
                            o, whh[:, gt * 128:(gt + 1) * 128], hsrc[:, 0, sp, :],
                            start=False, stop=False))
                        grp[g].append(nc.tensor.matmul(
                            o, whh[:, 768 + gt * 128:768 + (gt + 1) * 128],
                            hsrc[:, 1, sp, :], start=False, stop=(g_last := (j == 1))))
                for insts in grp.values():
                    for a, b in zip(insts, insts[1:]):
                        tile.add_dep_helper(b.ins, a.ins, False, "acc order")
                rt = rzp.tile([128, 32], wdt, tag="rt")
                nc.scalar.activation(rt[:], pr[:], Act.Sigmoid)
                npre = tmpp.tile([128, 32], wdt, tag="npre")
                nc.vector.tensor_mul(npre[:], pn[:], rt[:])
                ns = tmpp.tile([128, 32], wdt, tag="ns")
                ns_i = nc.vector.tensor_add(ns[:], npre[:], gi[:, s, :, :])
                nt = tmpp.tile([128, 32], wdt, tag="nt")
                nc.scalar.activation(nt[:], ns[:], Act.Tanh)
                zt = rzp.tile([128, 32], wdt, tag="zt")
                nc.scalar.activation(zt[:], pz[:], Act.Sigmoid)
                zh = tmpp.tile([128, 32], wdt, tag="zh")
                zh_i = nc.vector.tensor_mul(zh[:], zt[:], fsrc[:, :, sp, :])
                z2 = rzp.tile([128, 32], wdt, tag="z2")
                z2_i = nc.vector.tensor_scalar(z2[:], zt[:], -1.0, 1.0,
                                               op0=Alu.mult, op1=Alu.add)
                tile.add_dep_helper(zh_i.ins, ns_i.ins, False, "keep n-chain first")
                tile.add_dep_helper(z2_i.ins, zh_i.ins, False, "keep n-chain first")
                v = tmpp.tile([128, 32], wdt, tag="v")
                nc.vector.tensor_mul(v[:], z2[:], nt[:])
                nc.vector.tensor_add(curf[:, :, s, :], v[:], zh[:])
                if state_fp32:
                    nc.vector.tensor_copy(curh[:, :, s, :], curf[:, :, s, :])

        head_state = {}

        def emit_head_a(p, h8):
            # gain + h@w_bias matmuls and the gain sigmoid for one 128-sample
            # slice; the feat/policy half is emitted a few steps later so the
            # head fills two step-idle windows instead of overflowing one.
            xb, hh = xbuf[p], hbf[p]
            sl = slice(h8 * 128, (h8 + 1) * 128)
            ssl = slice(h8 * 8, (h8 + 1) * 8)
            pgain = phd.tile([2, 128], f32, tag="ph", name="pgain")
            nc.tensor.matmul(pgain[:], wgb[:, 0:2], hh[:, 0, ssl, :], start=True, stop=False)
            nc.tensor.matmul(pgain[:], wgb[:, 2:4], hh[:, 1, ssl, :], start=False, stop=True)
            ppm = phd.tile([34, 128], f32, tag="ph", name="ppm")
            nc.tensor.matmul(ppm[:], wbm[:, 0:34], hh[:, 0, ssl, :], start=True, stop=False)
            nc.tensor.matmul(ppm[:], wbm[:, 34:68], hh[:, 1, ssl, :], start=False, stop=False)
            gain = tmpp.tile([2, 128], wdt, tag="gain")
            nc.scalar.activation(gain[:], pgain[:], Act.Sigmoid, bias=bgain[:, 0:1])
            xg = tmpp.tile([2, 128], wdt, tag="xg")
            nc.vector.tensor_mul(xg[:], xb[0:2, sl], gain[:])
            head_state[(p, h8)] = (ppm, xg)

        def emit_head_b(p, h8):
            ppm, xg = head_state.pop((p, h8))
            ssl = slice(h8 * 8, (h8 + 1) * 8)
            feat = tmpp.tile([128, 2, 128], wdt, tag="feat")
            pf = phd.tile([128, 256], f32, tag="ph", name="pf")
            for mt in (0, 1):
                nc.tensor.matmul(pf[:, mt * 128:(mt + 1) * 128],
                                 wrx[:, mt * 128:(mt + 1) * 128], xg[:],
                                 start=(mt == 0), stop=False)
            for mt in (0, 1):
                nc.tensor.matmul(pf[:, mt * 128:(mt + 1) * 128],
                                 brx[0:1, mt * 128:(mt + 1) * 128], ones16[0:1, :],
                                 start=False, stop=(mt == 1))
            nc.scalar.activation(feat[:, :, :], pf[:], Act.Relu)
            nc.tensor.matmul(ppm[:], wpm[:, 0:34], feat[:, 0, :], start=False, stop=False)
            nc.tensor.matmul(ppm[:], wpm[:, 34:68], feat[:, 1, :], start=False, stop=True)
            nc.scalar.activation(ysp[p][:, ssl, :], ppm[0:2, :], Act.Identity, bias=bgb[:, 0:1])
            nc.scalar.activation(ysd[p][:, ssl, :], ppm[32:34, :], Act.Tanh, bias=bgb[:, 1:2])

        def emit_out(p, cc, nh):
            src = hstg[p]
            nc.sync.dma_start(h_d[:, :, ds(cc * CHUNK + nh * 32, 32), :],
                              src[:, :, nh * 32:(nh + 1) * 32, :])
            nc.sync.dma_start(y_d[0:2, ds(cc * CHUNK + nh * 32, 32), :],
                              ysp[p][:, nh * 32:(nh + 1) * 32, :])
            nc.sync.dma_start(y_d[2:4, ds(cc * CHUNK + nh * 32, 32), :],
                              ysd[p][:, nh * 32:(nh + 1) * 32, :])

        def emit_chunk(p, cc):
            nc.sync.dma_start(xbuf[1 - p][:], x_d[:, ts(cc + 1, CHUNK * BL)])
            pend = None
            for blk in range(8):
                for s in range(blk * 8, (blk + 1) * 8):
                    emit_steps(p, s, s + 1)
                    if s % 8 == 4 and pend is not None:
                        pb, pend = pend, None
                        emit_head_b(p, pb)
                        if pb == 3:
                            emit_out(p, cc, 0)
                emit_head_a(p, blk)
                pend = blk
                if blk % 2 == 1:
                    emit_gin(gin[1 - p], xbuf[1 - p], pieces=(blk // 2,))
            emit_head_b(p, 7)
            emit_out(p, cc, 1)

        emit_gin(gin[0], xbuf[0])
        per_body = 8 if NCH % 8 == 0 else 2
        with tc.For_i(0, NCH // per_body, 1,
                      hint_engines=(mybir.EngineType.PE, mybir.EngineType.DVE,
                                    mybir.EngineType.Activation)) as iv:
            for q in range(per_body):
                emit_chunk(q % 2, iv * per_body + q)

    nc.compile()
    return nc


def _get(S=S_FULL, state_fp32=True, debug=False):
    key = (S, state_fp32, debug)
    if key not in _CACHE:
        _CACHE[key] = _build(S, state_fp32, debug)
    return _CACHE[key]


def _prep_inputs(S, x, w_ih, w_hh, b_ih, b_hh, w_gain, b_gain, w_bias, b_bias,
                 w_reflex, b_reflex, w_policy, b_policy, w_motor, b_motor):
    """Host-side packing of the (replicated) weights and per-core x shards."""
    bfn = ml_dtypes.bfloat16
    f32 = np.float32
    NCH = S // CHUNK
    whh_t = np.asarray(w_hh, f32).T  # [256, 768]
    whh_arr = np.concatenate([whh_t[0:128, :], whh_t[128:256, :]], axis=1)
    wx_arr = np.empty((7, 768), f32)
    wx_arr[0:6] = np.asarray(w_ih, f32).T
    bsum = np.asarray(b_ih, f32) + np.asarray(b_hh, f32)
    wx_arr[6, 0:512] = bsum[0:512]
    wx_arr[6, 512:768] = np.asarray(b_ih, f32)[512:768]
    bhn_arr = np.asarray(b_hh, f32)[512:768].reshape(1, 256)
    wgb = np.asarray(w_gain, f32).T  # [256, 2]
    wgb_arr = np.concatenate([wgb[0:128, :], wgb[128:256, :]], axis=1)
    wbm = np.zeros((256, 34), f32)
    wbm[:, 0:2] = np.asarray(w_bias, f32).T[:, 0:2]
    wbm[:, 32:34] = np.asarray(w_bias, f32).T[:, 2:4]
    wbm_arr = np.concatenate([wbm[0:128, :], wbm[128:256, :]], axis=1)
    wrx_arr = np.asarray(w_reflex, f32).T.copy()  # [2,256]
    wpm = np.zeros((256, 34), f32)
    wpm[:, 0:2] = np.asarray(w_policy, f32).T
    wpm[:, 32:34] = np.asarray(w_motor, f32).T
    wpm_arr = np.concatenate([wpm[0:128, :], wpm[128:256, :]], axis=1)
    bgain_arr = np.asarray(b_gain, f32).reshape(2, 1)
    bgb_arr = np.stack([
        np.asarray(b_bias, f32)[0:2] + np.asarray(b_policy, f32),
        np.asarray(b_bias, f32)[2:4] + np.asarray(b_motor, f32)], axis=1)
    brx_arr = np.asarray(b_reflex, f32).reshape(1, 256)
    common = dict(
        whh=np.ascontiguousarray(whh_arr).astype(bfn),
        wx=wx_arr.astype(bfn), bhn=bhn_arr.astype(bfn),
        wgb=np.ascontiguousarray(wgb_arr).astype(bfn),
        wbm=np.ascontiguousarray(wbm_arr).astype(bfn),
        wrx=wrx_arr.astype(bfn),
        wpm=np.ascontiguousarray(wpm_arr).astype(bfn),
        bgain=bgain_arr, bgb=bgb_arr, brx=brx_arr.astype(bfn))
    x = np.asarray(x, f32)
    in_maps = []
    for c in range(NCORES):
        xc = x[c * BL:(c + 1) * BL, 0:S, :]  # [BL, S, 6]
        xt = np.zeros((7, (NCH + 1) * CHUNK, BL), f32)
        xt[0:6, 0:S, :] = xc.transpose(2, 1, 0)
        xt[6, 0:S, :] = 1.0
        in_maps.append({**common, "x": xt.reshape(7, -1).astype(bfn)})
    return in_maps


def _assemble(S, results):
    y_pred = np.empty((B, S, 4), np.float32)
    router = np.empty((B, S, H), np.float32)
    for c, r in enumerate(results):
        h = np.asarray(r["h_out"]).astype(np.float32)  # [128, 2, S, BL]
        router[c * BL:(c + 1) * BL] = h.transpose(3, 2, 1, 0).reshape(BL, S, H)
        y = np.asarray(r["y_out"]).astype(np.float32)  # [4, S, BL]
        y_pred[c * BL:(c + 1) * BL] = y.transpose(2, 1, 0)
    return y_pred, router


def run(inputs, S=S_FULL, state_fp32=False, trace=False, **run_kwargs):
    from concourse.bass_utils import run_bass_kernel_spmd
    nc = _get(S, state_fp32)
    in_maps = _prep_inputs(S, **inputs)
    res = run_bass_kernel_spmd(nc, in_maps, core_ids=list(range(NCORES)),
                               trace=trace, **run_kwargs)
    y_pred, router = _assemble(S, res.results)
    return (y_pred, router), res


def kernel(**inputs):
    (y_pred, router), _ = run(inputs, S=S_FULL, state_fp32=False, trace=False)
    return y_pred, router


# revision 25
# speedup vs baseline: 1.0052x; 1.0052x over previous
"""Trainium2 Bass kernel for nn_BioMoR_RNN (GRU recurrence + small output heads).

Strategy (pure data parallel, per the spec sharding hint):
  - batch 128 is split 16-per-core across 8 NeuronCores; all weights replicated.
  - The GRU recurrence runs gate-major: per step, 18 bf16 matmuls accumulate
    W_hh @ h plus the x/bias contributions into PSUM tiles [gate, batch] for
    r, z, n.  The x / bias matmuls have no h dependency and OPEN each PSUM
    accumulation group (start=True), so the scheduler executes them during the
    previous step's nonlinear chain; only the 12 W_hh matmuls sit on the
    critical path, issuing back-to-back (~27ns) with preloaded weights.
  - Chain per step: sigmoid(r) overlaps the n/z matmuls; n-path is
    mul(psum_n, r) -> +gi_n -> tanh; update is h' = (1-z)*n + z*h with
    z*h and (1-z) computed during the tanh window, leaving only two
    dependent vector ops after tanh.  All elementwise ops are bf16 (DVE 2x).
  - The n-gate x contribution (gi_n) is precomputed per 64-step chunk as
    batched K=7 matmuls; b_hh_n enters psum_n via a K=1 ones-row matmul.
  - Output heads are computed as batched matmuls over 128-sample slices,
    interleaved between every 8 steps so they fill engine idle time without
    head-of-line blocking the recurrence.  The h@w_bias head accumulates into
    the same PSUM tile as the policy/motor matmuls (at partition bases 0/32),
    so probs/dirs each collapse to one activation op with constant bias;
    b_reflex enters the feat psum via K=1 ones-matmuls (single wide relu).
  - h_t streams to HBM as [h, s, b] (gate-major, bf16); the host transposes
    and casts.  Dynamic loop over chunks (8 per body) keeps code size small.

Self-contained: hardcodes shapes from the problem spec; no file reads.
"""

import contextlib
import numpy as np
import ml_dtypes

B, S_FULL, H, DX = 128, 2048, 256, 6
NCORES = 8
BL = B // NCORES  # 16 batch rows per core
CHUNK = 64        # steps per chunk

_CACHE = {}


def _build(S=S_FULL, state_fp32=True, debug=False):
    """Build + schedule the per-core kernel (same NEFF for all 8 cores)."""
    import concourse.tile as tile
    from concourse import bacc, mybir
    from concourse.bass import ds, ts

    f32 = mybir.dt.float32
    bf16 = mybir.dt.bfloat16
    Alu = mybir.AluOpType
    Act = mybir.ActivationFunctionType
    wdt = bf16

    NCH = S // CHUNK
    assert S % CHUNK == 0 and NCH % 2 == 0
    hdt = f32 if state_fp32 else bf16

    nc = bacc.Bacc("TRN2", target_bir_lowering=False, debug=debug)

    x_d = nc.dram_tensor("x", [7, (NCH + 1) * CHUNK * BL], wdt, kind="ExternalInput")
    whh_d = nc.dram_tensor("whh", [128, 2 * 768], wdt, kind="ExternalInput")
    wx_d = nc.dram_tensor("wx", [7, 768], wdt, kind="ExternalInput")
    bhn_d = nc.dram_tensor("bhn", [1, 256], wdt, kind="ExternalInput")
    wgb_d = nc.dram_tensor("wgb", [128, 4], wdt, kind="ExternalInput")
    wbm_d = nc.dram_tensor("wbm", [128, 68], wdt, kind="ExternalInput")
    wrx_d = nc.dram_tensor("wrx", [2, 256], wdt, kind="ExternalInput")
    wpm_d = nc.dram_tensor("wpm", [128, 68], wdt, kind="ExternalInput")
    bgain_d = nc.dram_tensor("bgain", [2, 1], f32, kind="ExternalInput")
    bgb_d = nc.dram_tensor("bgb", [2, 2], f32, kind="ExternalInput")
    brx_d = nc.dram_tensor("brx", [1, 256], wdt, kind="ExternalInput")
    h_d = nc.dram_tensor("h_out", [128, 2, S, BL], hdt, kind="ExternalOutput")
    y_d = nc.dram_tensor("y_out", [4, S, BL], f32, kind="ExternalOutput")

    with tile.TileContext(nc) as tc, contextlib.ExitStack() as ctx:
        constp = ctx.enter_context(tc.tile_pool(name="const", bufs=1))
        przp = ctx.enter_context(tc.tile_pool(name="prz", bufs=2, space="PSUM"))
        pzp = ctx.enter_context(tc.tile_pool(name="pz", bufs=1, space="PSUM"))
        pnp = ctx.enter_context(tc.tile_pool(name="pn", bufs=2, space="PSUM"))
        phd = ctx.enter_context(tc.tile_pool(name="phd", bufs=2, space="PSUM"))
        pgi = ctx.enter_context(tc.tile_pool(name="pgi", bufs=1, space="PSUM"))
        rzp = ctx.enter_context(tc.tile_pool(name="rzp", bufs=4))
        tmpp = ctx.enter_context(tc.tile_pool(name="tmp", bufs=4))

        whh = constp.tile([128, 2 * 768], wdt, tag="whh")
        wx = constp.tile([7, 768], wdt, tag="wx")
        bhn = constp.tile([1, 256], wdt, tag="bhn")
        ones16 = constp.tile([1, 128], wdt, tag="ones16")
        wgb = constp.tile([128, 4], wdt, tag="wgb")
        wbm = constp.tile([128, 68], wdt, tag="wbm")
        wrx = constp.tile([2, 256], wdt, tag="wrx")
        wpm = constp.tile([128, 68], wdt, tag="wpm")
        bgain = constp.tile([2, 1], f32, tag="bgain")
        bgb = constp.tile([2, 2], f32, tag="bgb")
        brx = constp.tile([1, 256], wdt, tag="brx")
        hbf = [constp.tile([128, 2, CHUNK, BL], wdt, tag=f"hbf{i}", name=f"hbf{i}") for i in (0, 1)]
        if state_fp32:
            hstg = [constp.tile([128, 2, CHUNK, BL], f32, tag=f"hstg{i}", name=f"hstg{i}") for i in (0, 1)]
        else:
            hstg = hbf
        xbuf = [constp.tile([7, CHUNK * BL], wdt, tag=f"xb{i}", name=f"xb{i}") for i in (0, 1)]
        gin = [constp.tile([128, CHUNK, 2, BL], wdt, tag=f"gin{i}", name=f"gin{i}") for i in (0, 1)]
        ysp = [constp.tile([2, CHUNK, BL], f32, tag=f"ysp{i}", name=f"ysp{i}") for i in (0, 1)]
        ysd = [constp.tile([2, CHUNK, BL], f32, tag=f"ysd{i}", name=f"ysd{i}") for i in (0, 1)]

        for tdst, tsrc in [(whh, whh_d), (wx, wx_d), (bhn, bhn_d), (wgb, wgb_d),
                           (wrx, wrx_d), (wpm, wpm_d), (wbm, wbm_d), (bgain, bgain_d),
                           (bgb, bgb_d), (brx, brx_d)]:
            nc.sync.dma_start(tdst[:], tsrc[:])
        nc.sync.dma_start(xbuf[0][:], x_d[:, 0:CHUNK * BL])
        nc.vector.memset(ones16[:], 1.0)
        nc.vector.memset(hbf[1][:, :, CHUNK - 1, :], 0.0)
        if state_fp32:
            nc.vector.memset(hstg[1][:, :, CHUNK - 1, :], 0.0)

        def emit_gin(dst, xsrc, pieces=(0, 1, 2, 3)):
            # n-gate x contribution (incl. b_ih_n via the ones row)
            for q in pieces:
                for mt in (0, 1):
                    pg = pgi.tile([128, 256], f32, tag="pg")
                    nc.tensor.matmul(pg[:], wx[:, (4 + mt) * 128:(5 + mt) * 128],
                                     xsrc[:, q * 256:(q + 1) * 256],
                                     start=True, stop=True)
                    nc.vector.tensor_copy(dst[:, q * 16:(q + 1) * 16, mt, :], pg[:])

        def emit_steps(p, s0, s1):
            curh, prevh = hbf[p], hbf[1 - p]
            curf, prevf = hstg[p], hstg[1 - p]
            xb, gi = xbuf[p], gin[p]
            for s in range(s0, s1):
                hsrc = curh if s else prevh
                fsrc = curf if s else prevf
                sp = s - 1 if s else CHUNK - 1
                pr = przp.tile([128, 32], f32, tag="pr", name="pr")
                pz = pzp.tile([128, 32], f32, tag="pz", name="pz")
                pn = pnp.tile([128, 32], f32, tag="pn", name="pn")
                # One accumulation group per PSUM tile: the x/bias matmul opens
                # the group (start=True clears the bank) and has no h dependency,
                # so the scheduler runs it during the previous step's chain.
                grp = {"r": [], "n": [], "z": []}
                for j, gt in enumerate((0, 1)):
                    grp["r"].append(nc.tensor.matmul(
                        pr[:, j * BL:(j + 1) * BL], wx[:, gt * 128:(gt + 1) * 128],
                        xb[:, s * BL:(s + 1) * BL], start=(j == 0), stop=False))
                for j in (0, 1):
                    grp["n"].append(nc.tensor.matmul(
                        pn[:, j * BL:(j + 1) * BL], bhn[0:1, j * 128:(j + 1) * 128],
                        ones16[0:1, 0:BL], start=(j == 0), stop=False))
                for j, gt in enumerate((2, 3)):
                    grp["z"].append(nc.tensor.matmul(
                        pz[:, j * BL:(j + 1) * BL], wx[:, gt * 128:(gt + 1) * 128],
                        xb[:, s * BL:(s + 1) * BL], start=(j == 0), stop=False))
                # h matmuls: r first (sigmoid(r) overlaps), then n, then z
                for g, base, pt in (("r", 0, pr), ("n", 4, pn), ("z", 2, pz)):
                    for j in (0, 1):
                        gt = base + j
                        o = pt[:, j * BL:(j + 1) * BL]
                        grp[g].append(nc.tensor.matmul(
                            o, whh[:, gt * 128:(gt + 1) * 128], hsrc[:, 0, sp, :],
                            start=False, stop=False))
                        grp[g].append(nc.tensor.matmul(
                            o, whh[:, 768 + gt * 128:768 + (gt + 1) * 128],
                            hsrc[:, 1, sp, :], start=False, stop=(g_last := (j == 1))))
                for insts in grp.values():
                    for a, b in zip(insts, insts[1:]):
                        tile.add_dep_helper(b.ins, a.ins, False, "acc order")
                rt = rzp.tile([128, 32], wdt, tag="rt")
                nc.scalar.activation(rt[:], pr[:], Act.Sigmoid)
                npre = tmpp.tile([128, 32], wdt, tag="npre")
                nc.vector.tensor_mul(npre[:], pn[:], rt[:])
                ns = tmpp.tile([128, 32], wdt, tag="ns")
                ns_i = nc.vector.tensor_add(ns[:], npre[:], gi[:, s, :, :])
                nt = tmpp.tile([128, 32], wdt, tag="nt")
                nc.scalar.activation(nt[:], ns[:], Act.Tanh)
                zt = rzp.tile([128, 32], wdt, tag="zt")
                nc.scalar.activation(zt[:], pz[:], Act.Sigmoid)
                zh = tmpp.tile([128, 32], wdt, tag="zh")
                zh_i = nc.vector.tensor_mul(zh[:], zt[:], fsrc[:, :, sp, :])
                z2 = rzp.tile([128, 32], wdt, tag="z2")
                z2_i = nc.vector.tensor_scalar(z2[:], zt[:], -1.0, 1.0,
                                               op0=Alu.mult, op1=Alu.add)
                tile.add_dep_helper(zh_i.ins, ns_i.ins, False, "keep n-chain first")
                tile.add_dep_helper(z2_i.ins, zh_i.ins, False, "keep n-chain first")
                v = tmpp.tile([128, 32], wdt, tag="v")
                nc.vector.tensor_mul(v[:], z2[:], nt[:])
                nc.vector.tensor_add(curf[:, :, s, :], v[:], zh[:])
                if state_fp32:
                    nc.vector.tensor_copy(curh[:, :, s, :], curf[:, :, s, :])

        def emit_head(p, h8):
            # one 128-sample slice; policy/motor psum also accumulates the
            # h@w_bias head, so probs/dirs are single ACT ops with const bias
            xb, hh = xbuf[p], hbf[p]
            sl = slice(h8 * 128, (h8 + 1) * 128)
            ssl = slice(h8 * 8, (h8 + 1) * 8)
            pgain = phd.tile([2, 128], f32, tag="ph", name="pgain")
            nc.tensor.matmul(pgain[:], wgb[:, 0:2], hh[:, 0, ssl, :], start=True, stop=False)
            nc.tensor.matmul(pgain[:], wgb[:, 2:4], hh[:, 1, ssl, :], start=False, stop=True)
            ppm = phd.tile([34, 128], f32, tag="ph", name="ppm")
            nc.tensor.matmul(ppm[:], wbm[:, 0:34], hh[:, 0, ssl, :], start=True, stop=False)
            nc.tensor.matmul(ppm[:], wbm[:, 34:68], hh[:, 1, ssl, :], start=False, stop=False)
            gain = tmpp.tile([2, 128], wdt, tag="gain")
            nc.scalar.activation(gain[:], pgain[:], Act.Sigmoid, bias=bgain[:, 0:1])
            xg = tmpp.tile([2, 128], wdt, tag="xg")
            nc.vector.tensor_mul(xg[:], xb[0:2, sl], gain[:])
            feat = tmpp.tile([128, 2, 128], wdt, tag="feat")
            pf = phd.tile([128, 256], f32, tag="ph", name="pf")
            for mt in (0, 1):
                nc.tensor.matmul(pf[:, mt * 128:(mt + 1) * 128],
                                 wrx[:, mt * 128:(mt + 1) * 128], xg[:],
                                 start=(mt == 0), stop=False)
            for mt in (0, 1):
                nc.tensor.matmul(pf[:, mt * 128:(mt + 1) * 128],
                                 brx[0:1, mt * 128:(mt + 1) * 128], ones16[0:1, :],
                                 start=False, stop=(mt == 1))
            nc.scalar.activation(feat[:, :, :], pf[:], Act.Relu)
            nc.tensor.matmul(ppm[:], wpm[:, 0:34], feat[:, 0, :], start=False, stop=False)
            nc.tensor.matmul(ppm[:], wpm[:, 34:68], feat[:, 1, :], start=False, stop=True)
            nc.scalar.activation(ysp[p][:, ssl, :], ppm[0:2, :], Act.Identity, bias=bgb[:, 0:1])
            nc.scalar.activation(ysd[p][:, ssl, :], ppm[32:34, :], Act.Tanh, bias=bgb[:, 1:2])

        def emit_out(p, cc, nh):
            src = hstg[p]
            nc.sync.dma_start(h_d[:, :, ds(cc * CHUNK + nh * 32, 32), :],
                              src[:, :, nh * 32:(nh + 1) * 32, :])
            nc.sync.dma_start(y_d[0:2, ds(cc * CHUNK + nh * 32, 32), :],
                              ysp[p][:, nh * 32:(nh + 1) * 32, :])
            nc.sync.dma_start(y_d[2:4, ds(cc * CHUNK + nh * 32, 32), :],
                              ysd[p][:, nh * 32:(nh + 1) * 32, :])

        def emit_chunk(p, cc):
            nc.sync.dma_start(xbuf[1 - p][:], x_d[:, ts(cc + 1, CHUNK * BL)])
            for blk in range(8):
                emit_steps(p, blk * 8, blk * 8 + 4)
                if blk % 2 == 1:
                    # gi for the next chunk, mid-block so it does not stack
                    # onto the head-piece boundary steps
                    emit_gin(gin[1 - p], xbuf[1 - p], pieces=(blk // 2,))
                emit_steps(p, blk * 8 + 4, (blk + 1) * 8)
                emit_head(p, blk)
                if blk == 3:
                    emit_out(p, cc, 0)
            emit_out(p, cc, 1)

        emit_gin(gin[0], xbuf[0])
        per_body = 8 if NCH % 8 == 0 else 2
        with tc.For_i(0, NCH // per_body, 1,
                      hint_engines=(mybir.EngineType.PE, mybir.EngineType.DVE,
                                    mybir.EngineType.Activation)) as iv:
            for q in range(per_body):
                emit_chunk(q % 2, iv * per_body + q)

    nc.compile()
    return nc


def _get(S=S_FULL, state_fp32=True, debug=False):
    key = (S, state_fp32, debug)
    if key not in _CACHE:
        _CACHE[key] = _build(S, state_fp32, debug)
    return _CACHE[key]


def _prep_inputs(S, x, w_ih, w_hh, b_ih, b_hh, w_gain, b_gain, w_bias, b_bias,
                 w_reflex, b_reflex, w_policy, b_policy, w_motor, b_motor):
    """Host-side packing of the (replicated) weights and per-core x shards."""
    bfn = ml_dtypes.bfloat16
    f32 = np.float32
    NCH = S // CHUNK
    whh_t = np.asarray(w_hh, f32).T  # [256, 768]
    whh_arr = np.concatenate([whh_t[0:128, :], whh_t[128:256, :]], axis=1)
    wx_arr = np.empty((7, 768), f32)
    wx_arr[0:6] = np.asarray(w_ih, f32).T
    bsum = np.asarray(b_ih, f32) + np.asarray(b_hh, f32)
    wx_arr[6, 0:512] = bsum[0:512]
    wx_arr[6, 512:768] = np.asarray(b_ih, f32)[512:768]
    bhn_arr = np.asarray(b_hh, f32)[512:768].reshape(1, 256)
    wgb = np.asarray(w_gain, f32).T  # [256, 2]
    wgb_arr = np.concatenate([wgb[0:128, :], wgb[128:256, :]], axis=1)
    wbm = np.zeros((256, 34), f32)
    wbm[:, 0:2] = np.asarray(w_bias, f32).T[:, 0:2]
    wbm[:, 32:34] = np.asarray(w_bias, f32).T[:, 2:4]
    wbm_arr = np.concatenate([wbm[0:128, :], wbm[128:256, :]], axis=1)
    wrx_arr = np.asarray(w_reflex, f32).T.copy()  # [2,256]
    wpm = np.zeros((256, 34), f32)
    wpm[:, 0:2] = np.asarray(w_policy, f32).T
    wpm[:, 32:34] = np.asarray(w_motor, f32).T
    wpm_arr = np.concatenate([wpm[0:128, :], wpm[128:256, :]], axis=1)
    bgain_arr = np.asarray(b_gain, f32).reshape(2, 1)
    bgb_arr = np.stack([
        np.asarray(b_bias, f32)[0:2] + np.asarray(b_policy, f32),
        np.asarray(b_bias, f32)[2:4] + np.asarray(b_motor, f32)], axis=1)
    brx_arr = np.asarray(b_reflex, f32).reshape(1, 256)
    common = dict(
        whh=np.ascontiguousarray(whh_arr).astype(bfn),
        wx=wx_arr.astype(bfn), bhn=bhn_arr.astype(bfn),
        wgb=np.ascontiguousarray(wgb_arr).astype(bfn),
        wbm=np.ascontiguousarray(wbm_arr).astype(bfn),
        wrx=wrx_arr.astype(bfn),
        wpm=np.ascontiguousarray(wpm_arr).astype(bfn),
        bgain=bgain_arr, bgb=bgb_arr, brx=brx_arr.astype(bfn))
    x = np.asarray(x, f32)
    in_maps = []
    for c in range(NCORES):
        xc = x[c * BL:(c + 1) * BL, 0:S, :]  # [BL, S, 6]
        xt = np.zeros((7, (NCH + 1) * CHUNK, BL), f32)
        xt[0:6, 0:S, :] = xc.transpose(2, 1, 0)
        xt[6, 0:S, :] = 1.0
        in_maps.append({**common, "x": xt.reshape(7, -1).astype(bfn)})
    return in_maps


def _assemble(S, results):
    y_pred = np.empty((B, S, 4), np.float32)
    router = np.empty((B, S, H), np.float32)
    for c, r in enumerate(results):
        h = np.asarray(r["h_out"]).astype(np.float32)  # [128, 2, S, BL]
        router[c * BL:(c + 1) * BL] = h.transpose(3, 2, 1, 0).reshape(BL, S, H)
        y = np.asarray(r["y_out"]).astype(np.float32)  # [4, S, BL]
        y_pred[c * BL:(c + 1) * BL] = y.transpose(2, 1, 0)
    return y_pred, router


def run(inputs, S=S_FULL, state_fp32=False, trace=False, **run_kwargs):
    from concourse.bass_utils import run_bass_kernel_spmd
    nc = _get(S, state_fp32)
    in_maps = _prep_inputs(S, **inputs)
    res = run_bass_kernel_spmd(nc, in_maps, core_ids=list(range(NCORES)),
                               trace=trace, **run_kwargs)
    y_pred, router = _assemble(S, res.results)
    return (y_pred, router), res


def kernel(**inputs):
    (y_pred, router), _ = run(inputs, S=S_FULL, state_fp32=False, trace=False)
    return y_pred, router
